# revision 42
# baseline (speedup 1.0000x reference)
"""3-layer GraphSAGE (ClusterGCN-style) on 8 Trainium2 NeuronCores.

Strategy (graph/data parallel, transform-first):
  - Nodes sharded by contiguous range across 8 cores (6250 each).
  - Per layer l: t = h @ Wl computed on own shard (fused into the previous
    layer's per-block epilogue) -> AllGather t -> per 128-dst block:
    dma_gather the incoming edges' t[src] rows -> segment-sum via one-hot
    matmul on the tensor engine -> combine with the root path h @ Wr + b
    (precomputed into SBUF during the AllGather window, when every engine
    would otherwise idle) -> relu -> PE transposes back to feature-major,
    in place in the single h table.
  - Layers 0/1 tables are fp8e4 (halves the AllGather bytes and the gather
    traffic; 512B rows keep full DMA rate). Aggregation uses pure-0/1 fp8
    one-hot S with MatmulPerfMode.DoubleRow over chunk pairs (256-deep
    contraction at 0.5 cycles/row); deg_inv is applied exactly via a
    per-partition fp32 scale on the PSUM->SBUF copy, and the root is added
    with an identity matmul.
  - Layer 2 (64-wide) packs 4 nodes per 256B fp8e3 table row (values
    pre-scaled by 8 to stay in e3m4's normal range), satisfying
    dma_gather's 256B row-alignment with a 4x smaller table; chunk class
    src%4 selects the 64-col slice of each gathered row.
  - Edges are dst-sorted on host into 128-edge chunks per (block, class)
    with static chunk counts (max over cores -> one SPMD program); gathers
    and the S build are batched over pairs of blocks and spread across 4
    SWDGE queues; the per-block epilogue is software-pipelined one block
    behind the aggregation.
  - fp32 PSUM accumulation everywhere; fp32 output.
"""

import math
import numpy as np

N_NODES = 50000
N_EDGES = 800000
D_IN = 512
D_HID = 512
D_OUT = 64
N_CORES = 8
LOW_LIM_FULL = 32768


# ---------------------------------------------------------------------------
# Host preprocessing
# ---------------------------------------------------------------------------

class Plan:
    pass


def _wrap_idx(v):
    """Pack an index vector (len multiple of 16) into the [16, m/16]
    pattern dma_gather expects, replicated to 128 partitions."""
    a = np.asarray(v, np.int16).reshape(-1, 16).T  # [16, m/16]
    return np.tile(a, (8, 1))  # [128, m/16]


def preprocess(x, edge_index, n_nodes, n_cores, d_in, low_lim):
    """Returns (plan, per_core_inputs_list)."""
    src = np.asarray(edge_index[0], np.int64)
    dst = np.asarray(edge_index[1], np.int64)
    nsh = n_nodes // n_cores
    nblk = math.ceil(nsh / 128)
    nfree = nblk * 128
    kc = d_in // 128

    deg = np.bincount(dst, minlength=n_nodes).astype(np.float32)
    deginv = (1.0 / np.maximum(deg, 1.0)).astype(np.float32)

    core = dst // nsh
    bid = core * nblk + (dst - core * nsh) // 128
    cls = (src >= low_lim).astype(np.int64)
    order = np.lexsort((dst, cls, bid))
    src_s = src[order]
    dst_s = dst[order]
    bid_s = bid[order]
    cls_s = cls[order]

    key = bid_s * 2 + cls_s
    ngrp = n_cores * nblk * 2
    starts = np.searchsorted(key, np.arange(ngrp + 1), side="left")

    # static chunk counts per (block, class): max over cores
    cnt = (starts[1:] - starts[:-1]).reshape(n_cores, nblk, 2)
    nchunk = -(-cnt // 128)  # ceil
    nL = nchunk[:, :, 0].max(axis=0)  # [nblk]
    nH = nchunk[:, :, 1].max(axis=0)
    CL = int(nL.sum())
    CH = int(nH.sum())
    TOTC = CL + CH
    offL = np.concatenate([[0], np.cumsum(nL)[:-1]]).astype(np.int64)
    offH = np.concatenate([[0], np.cumsum(nH)[:-1]]).astype(np.int64)
    offT = np.concatenate([[0], np.cumsum(nL + nH)[:-1]]).astype(np.int64)

    plan = Plan()
    plan.nsh, plan.nblk, plan.nfree, plan.kc = nsh, nblk, nfree, kc
    plan.nL, plan.nH = nL.tolist(), nH.tolist()
    plan.CL, plan.CH, plan.TOTC = CL, CH, TOTC
    plan.offL, plan.offH, plan.offT = offL.tolist(), offH.tolist(), offT.tolist()
    plan.low_lim = low_lim
    plan.n_cores = n_cores
    plan.n_nodes = n_nodes

    # ---- layer-2 packed-table structure: 4 nodes per 256B fp8 row, so the
    # 64-wide final layer satisfies dma_gather's 256B alignment with a table
    # 4x smaller. Chunk class = src%4 selects the 64-col slice of the row.
    core_src = src // nsh
    ls = src - core_src * nsh
    row4 = core_src * (nfree // 4) + ls // 4  # row in the [8*nfree/4, 256] view
    cls4 = ls % 4
    order2 = np.lexsort((dst, cls4, bid))
    row_s2 = row4[order2]
    dst_s2 = dst[order2]
    cls_s2 = cls4[order2]
    bid_s2 = bid[order2]
    key2 = bid_s2 * 4 + cls_s2
    starts2 = np.searchsorted(key2, np.arange(n_cores * nblk * 4 + 1), "left")
    cnt2 = (starts2[1:] - starts2[:-1]).reshape(n_cores, nblk, 4)
    n4 = (-(-cnt2 // 128)).max(axis=0)  # [nblk, 4]
    TOT2 = int(n4.sum())
    offB2 = np.concatenate([[0], np.cumsum(n4.sum(axis=1))[:-1]]).astype(np.int64)
    plan.n4 = n4.tolist()
    plan.TOT2 = TOT2
    plan.offB2 = offB2.tolist()
    plan.cls2 = [sum(([c] * int(n4[b, c]) for c in range(4)), [])
                 for b in range(nblk)]

    x = np.asarray(x, np.float32)
    per_core = []
    for c in range(n_cores):
        # layer-2 packed-table chunk inputs
        idx2 = np.zeros((TOT2 * 128,), np.int64)
        dloc2 = np.full((TOT2 * 128,), -1.0, np.float32)
        for b in range(nblk):
            pos = offB2[b]
            for cl in range(4):
                g = (c * nblk + b) * 4 + cl
                s0, s1 = starts2[g], starts2[g + 1]
                n_e = s1 - s0
                nX = n4[b][cl]
                if nX:
                    seg = idx2[pos * 128:(pos + nX) * 128]
                    seg[:n_e] = row_s2[s0:s1]
                    dseg = dloc2[pos * 128:(pos + nX) * 128]
                    dseg[:n_e] = (dst_s2[s0:s1] - c * nsh - b * 128).astype(
                        np.float32)
                    pos += nX
        wi_2 = _wrap_idx(idx2) if TOT2 else np.zeros((128, 0), np.int16)
        dl2 = dloc2.reshape(TOT2, 128).T.astype(np.float16)  # [128, TOT2]

        idxL = np.zeros((CL * 128,), np.int64)
        idxH = np.zeros((CH * 128,), np.int64)
        dloc = np.full((TOTC * 128,), -1.0, np.float32)
        for b in range(nblk):
            for t, (nX, offX, idxX, base) in enumerate(
                ((nL[b], offL[b], idxL, 0), (nH[b], offH[b], idxH, low_lim))
            ):
                g = (c * nblk + b) * 2 + t
                s0, s1 = starts[g], starts[g + 1]
                n_e = s1 - s0
                if nX == 0:
                    continue
                seg = idxX[offX * 128:(offX + nX) * 128]
                seg[:n_e] = src_s[s0:s1] - base
                # dloc columns in gather-group order (pairs of blocks):
                # [L(b0) L(b1) H(b0) H(b1)]
                b0 = b - (b % 2)
                b1 = b0 + 1
                nLb1 = nL[b1] if b1 < nblk else 0
                if t == 0:
                    dof = (offT[b0] + (0 if b == b0 else nL[b0])) * 128
                else:
                    dof = (offT[b0] + nL[b0] + nLb1
                           + (0 if b == b0 else nH[b0])) * 128
                dseg = dloc[dof:dof + nX * 128]
                dseg[:n_e] = (dst_s[s0:s1] - c * nsh - b * 128).astype(np.float32)
        wi_L = _wrap_idx(idxL) if CL else np.zeros((128, 0), np.int16)
        wi_H = _wrap_idx(idxH) if CH else np.zeros((128, 0), np.int16)
        # dloc layout must match gather output: edge i -> partition i%128,
        # chunk i//128; dstloc[p, chunk] = dloc of that edge
        dl = dloc.reshape(TOTC, 128).T.astype(np.float16)  # [128, TOTC]

        # per-partition deginv columns: dg_b[p, b] = deginv of node b*128+p
        dg = np.zeros((nfree,), np.float32)
        dg[:nsh] = deginv[c * nsh:(c + 1) * nsh]
        dg_b = np.ascontiguousarray(
            dg.reshape(nblk, 128).T.astype(np.float32))  # [128, nblk]

        xT = np.zeros((kc, 128, nfree), np.float16)
        xs = x[c * nsh:(c + 1) * nsh]  # [nsh, d_in]
        xT[:, :, :nsh] = xs.T.reshape(kc, 128, nsh)

        per_core.append({
            "xT": xT,
            "idxL": np.ascontiguousarray(wi_L),
            "idxH": np.ascontiguousarray(wi_H),
            "dstloc": np.ascontiguousarray(dl),
            "deginv": dg_b,
            "idx2": np.ascontiguousarray(wi_2),
            "dstloc2": np.ascontiguousarray(dl2),
            "deginv8": dg_b / 8.0,
        })
    return plan, per_core


# ---------------------------------------------------------------------------
# Device program
# ---------------------------------------------------------------------------

def build_program(plan, d_in, d_hid, d_out):
    import os
    dbg = set(os.environ.get("KDBG", "").split(",")) - {""}
    import concourse.bacc as bacc
    import concourse.tile as tile
    from concourse import bass, mybir
    from concourse.masks import make_identity

    f16 = mybir.dt.float16
    f32 = mybir.dt.float32
    f8 = mybir.dt.float8e4
    f8e3 = mybir.dt.float8e3
    i16 = mybir.dt.int16

    nsh, nblk, nfree = plan.nsh, plan.nblk, plan.nfree
    kcs = [d_in // 128, d_hid // 128, d_hid // 128]
    douts = [d_hid, d_hid, d_out]
    n_cores = plan.n_cores
    n_nodes = plan.n_nodes
    CL, CH, TOTC = plan.CL, plan.CH, plan.TOTC
    low_lim = plan.low_lim
    kc0 = kcs[0]

    nc = bacc.Bacc(
        "TRN2",
        target_bir_lowering=False,
        debug=False,
        num_devices=n_cores,
        num_swdge_queues=4,
    )

    xT_d = nc.dram_tensor("xT", [kc0, 128, nfree], f16, kind="ExternalInput").ap()
    w_d = {}
    for l in range(3):
        kd = [d_in, d_hid, d_hid][l]
        w_d[(l, "l")] = nc.dram_tensor(f"wl{l}", [kd, douts[l]], f16,
                                       kind="ExternalInput").ap()
        w_d[(l, "r")] = nc.dram_tensor(f"wr{l}", [kd, douts[l]], f16,
                                       kind="ExternalInput").ap()
    b_d = [nc.dram_tensor(f"b{l}", [1, douts[l]], f16, kind="ExternalInput").ap()
           for l in range(3)]
    idxL_d = nc.dram_tensor("idxL", [128, max(CL * 8, 1)], i16,
                            kind="ExternalInput").ap()
    idxH_d = nc.dram_tensor("idxH", [128, max(CH * 8, 1)], i16,
                            kind="ExternalInput").ap()
    dstloc_d = nc.dram_tensor("dstloc", [128, TOTC], f16, kind="ExternalInput").ap()
    deginv_d = nc.dram_tensor("deginv", [128, nblk], f32, kind="ExternalInput").ap()
    TOT2 = plan.TOT2
    idx2_d = nc.dram_tensor("idx2", [128, max(TOT2 * 8, 1)], i16,
                            kind="ExternalInput").ap()
    dstloc2_d = nc.dram_tensor("dstloc2", [128, TOT2], f16,
                               kind="ExternalInput").ap()
    deginv8_d = nc.dram_tensor("deginv8", [128, nblk], f32,
                               kind="ExternalInput").ap()
    out_d = nc.dram_tensor("out", [nsh, d_out], f32, kind="ExternalOutput").ap()

    with tile.TileContext(nc) as tc:
        # --- DRAM bounce buffers for the AllGathers
        ag_in, ag_out = [], []
        for l in range(3):
            if l < 2:
                ai = nc.dram_tensor(f"agi{l}", [nsh, douts[l]], f8,
                                    kind="Internal").ap()
                ao = nc.dram_tensor(f"ago{l}", [n_nodes, douts[l]], f8,
                                    kind="Internal", addr_space="Shared").ap()
            else:
                # packed: 4 nodes per 256B row (viewed [nfree,64] for writes,
                # [n_cores*nfree//4, 256] for gathers); values pre-scaled by 8
                # so fp8e3 stays in its normal range
                ai = nc.dram_tensor(f"agi{l}", [nfree, douts[l]], f8e3,
                                    kind="Internal").ap()
                ao = nc.dram_tensor(f"ago{l}", [n_cores * nfree, douts[l]],
                                    f8e3, kind="Internal",
                                    addr_space="Shared").ap()
            ag_in.append(ai)
            ag_out.append(ao)

        import contextlib
        with contextlib.ExitStack() as ctx:
            nb = 1 if "serial" in dbg else 3
            cpool = ctx.enter_context(tc.tile_pool(name="const", bufs=1))
            pt_pool = ctx.enter_context(
                tc.tile_pool(name="pt", bufs=2, space="PSUM"))
            pm_pool = ctx.enter_context(
                tc.tile_pool(name="pm", bufs=2, space="PSUM"))
            pma_pool = ctx.enter_context(
                tc.tile_pool(name="pma", bufs=2, space="PSUM"))
            tr_pool = ctx.enter_context(
                tc.tile_pool(name="tr", bufs=2, space="PSUM"))
            tsb_pool = ctx.enter_context(tc.tile_pool(name="tsb", bufs=nb))
            msg_pool = ctx.enter_context(tc.tile_pool(name="msg", bufs=2))
            s_pool = ctx.enter_context(tc.tile_pool(name="spool", bufs=3))
            a_pool = ctx.enter_context(tc.tile_pool(name="apool", bufs=3))
            h_pool = ctx.enter_context(tc.tile_pool(name="hpool", bufs=3))
            o_pool = ctx.enter_context(tc.tile_pool(name="opool", bufs=3))

            # --- constants
            # single in-place h table: layer l+1's rows overwrite layer l's
            # block by block after the root path has consumed them
            hT = cpool.tile([128, kc0 * nfree], f16, name="hT")
            hT3 = hT[:].rearrange("p (q n) -> p q n", n=nfree)
            xT3 = xT_d.rearrange("q p n -> p q n")
            step = (nblk // 4) * 128
            for c0 in range(0, nfree, step):
                c1 = min(c0 + step, nfree)
                nc.sync.dma_start(hT3[:, :, c0:c1], xT3[:, :, c0:c1])
            # root-path results (h @ Wr + b), staged in SBUF so the PE can
            # fill the AllGather window with this work
            rootS = cpool.tile([128, nblk * max(douts)], f16, name="rootS")

            ident = cpool.tile([128, 128], f16, name="ident")
            make_identity(nc, ident[:])
            iota = cpool.tile([128, 128], f16, name="iota")
            nc.gpsimd.iota(iota[:], pattern=[[1, 128]], base=0,
                           channel_multiplier=0,
                           allow_small_or_imprecise_dtypes=True)
            ones = cpool.tile([1, 128], f16, name="ones")
            nc.vector.memset(ones[:], 1.0)

            wt = {}
            for l in range(3):
                kd = kcs[l]
                for s in ("l", "r"):
                    t = cpool.tile([128, kd * douts[l]], f16, name=f"w{s}{l}")
                    nc.sync.dma_start(
                        t[:].rearrange("p (q d) -> p q d", d=douts[l]),
                        w_d[(l, s)].rearrange("(q p) d -> p q d", p=128))
                    wt[(l, s)] = t
            bt = []
            for l in range(3):
                t = cpool.tile([1, douts[l]], f16, name=f"bt{l}")
                nc.sync.dma_start(t[:], b_d[l][:, :])
                bt.append(t)

            IDXW = max(CL * 8 + CH * 8, TOT2 * 8, 1)
            idxU_t = cpool.tile([128, IDXW], i16, name="idxUt")
            nc.sync.dma_start(idxU_t[:, :CL * 8], idxL_d[:, :])
            nc.sync.dma_start(idxU_t[:, CL * 8:CL * 8 + CH * 8], idxH_d[:, :])
            dstloc_t = cpool.tile([128, TOTC], f16, name="dstloct")
            nc.sync.dma_start(dstloc_t[:], dstloc_d[:, :])
            deginv_t = cpool.tile([128, nblk], f32, name="deginvt")
            nc.sync.dma_start(deginv_t[:], deginv_d[:, :])
            dstloc2_t = cpool.tile([128, TOT2], f16, name="dstloc2t")
            nc.sync.dma_start(dstloc2_t[:], dstloc2_d[:, :])
            deginv8_t = cpool.tile([128, nblk], f32, name="deginv8t")
            nc.sync.dma_start(deginv8_t[:], deginv8_d[:, :])

            rg = [list(range(n_cores))]
            from concourse import mybir as _mb

            def phase1_block(l, b, pool=None):
                """t = h @ Wl for one block -> ag_in[l] (feeds the AllGather)."""
                dout = douts[l]
                kc = kcs[l]
                tdt = f8 if l < 2 else f8e3
                bs = slice(b * 128, (b + 1) * 128)
                rows = min(128, nsh - b * 128)
                pool, ptag = pool or (pt_pool, "pt")
                pt = pool.tile([128, dout], f32, tag=ptag)
                for q in range(kc):
                    nc.tensor.matmul(
                        pt[:], lhsT=hT3[:, q, bs],
                        rhs=wt[(l, "l")][:, q * dout:(q + 1) * dout],
                        start=(q == 0), stop=(q == kc - 1))
                tsb = tsb_pool.tile([128, dout], tdt, tag="tsb")
                if l < 2:
                    nc.scalar.copy(tsb[:], pt[:])
                    nc.sync.dma_start(ag_in[l][b * 128:b * 128 + rows, :],
                                      tsb[:rows, :])
                else:
                    nc.scalar.activation(tsb[:], pt[:],
                                         _mb.ActivationFunctionType.Copy,
                                         scale=8.0)
                    nc.sync.dma_start(ag_in[l][b * 128:(b + 1) * 128, :],
                                      tsb[:, :])

            # ---- layer 0 phase 1 (from the x table). The phase-3 PSUM
            # pools are idle here; rotating over them deepens the
            # PSUM->SBUF pipeline so the PE never waits on the copy-out.
            for b in range(nblk):
                phase1_block(0, b, [(pt_pool, "pt"), (pma_pool, "pma"),
                                    (pm_pool, "pm")][b % 3])

            for l in range(3):
                dout = douts[l]
                kc = kcs[l]
                tdt = f8 if l < 2 else f32

                # ---- AllGather t
                if "nocoll" not in dbg:
                    nc.gpsimd.collective_compute(
                        "AllGather", _mb.AluOpType.bypass, replica_groups=rg,
                        ins=[ag_in[l].opt()], outs=[ag_out[l].opt()])

                if l == 2:
                    # swap in the layer-2 gather indices (layers 0/1 are done
                    # with the union tile by now)
                    nc.sync.dma_start(idxU_t[:, :TOT2 * 8], idx2_d[:, :])

                # ---- root path (h @ Wr + b) -> rootS; no dependency on the
                # AllGather, so the PE runs this inside the collective window
                for b in range(nblk):
                    bs = slice(b * 128, (b + 1) * 128)
                    pmr = pt_pool.tile([128, dout], f32, tag="pt")
                    for q in range(kc):
                        nc.tensor.matmul(
                            pmr[:], lhsT=hT3[:, q, bs],
                            rhs=wt[(l, "r")][:, q * dout:(q + 1) * dout],
                            start=(q == 0), stop=False)
                    nc.tensor.matmul(pmr[:], lhsT=ones[:1, :], rhs=bt[l][:1, :],
                                     start=False, stop=True)
                    nc.scalar.copy(rootS[:, b * dout:(b + 1) * dout], pmr[:])

                # ---- phase 3, software-pipelined by one block:
                # head(b) = gathers/S/agg/combine, tail(b) = relu/transpose/
                # fused next phase1 (or output store). tail(b-1) is emitted
                # after head(b) so no engine queue waits on the relu round
                # trip between PE and Act.
                def gather_group(g):
                    """One msg tile + batched gathers + one S build covering
                    the pair of blocks (2g, 2g+1)."""
                    b0 = 2 * g
                    nb2 = 2 if b0 + 1 < nblk else 1
                    if l < 2:
                        nL0 = plan.nL[b0]
                        nH0 = plan.nH[b0]
                        nL1 = plan.nL[b0 + 1] if nb2 == 2 else 0
                        nH1 = plan.nH[b0 + 1] if nb2 == 2 else 0
                        nLg = nL0 + nL1
                        nHg = nH0 + nH1
                        gT = nLg + nHg
                        msg = msg_pool.tile([128, gT * dout], tdt, tag="msg")
                        msg3 = msg[:].rearrange("p (c e) -> p c e", e=dout)
                        if nLg:
                            nc.gpsimd.dma_gather(
                                msg3[:, :nLg, :], ag_out[l][:, :],
                                idxU_t[:, plan.offL[b0] * 8:(plan.offL[b0] + nLg) * 8],
                                num_idxs=nLg * 128, num_idxs_reg=nLg * 128,
                                elem_size=dout, single_packet=False,
                                queue_num=(2 * g) % 4)
                        if nHg:
                            nc.gpsimd.dma_gather(
                                msg3[:, nLg:, :], ag_out[l][low_lim:, :],
                                idxU_t[:, (CL + plan.offH[b0]) * 8:(CL + plan.offH[b0] + nHg) * 8],
                                num_idxs=nHg * 128, num_idxs_reg=nHg * 128,
                                elem_size=dout, single_packet=False,
                                queue_num=(2 * g + 1) % 4)
                        sdt = f8
                        doff = plan.offT[b0]
                        # per-block chunk runs in the group msg layout
                        runs = {b0: [(0, nL0), (nLg, nH0)]}
                        if nb2 == 2:
                            runs[b0 + 1] = [(nL0, nL1), (nLg + nH0, nH1)]
                    else:
                        n0 = sum(plan.n4[b0])
                        n1 = sum(plan.n4[b0 + 1]) if nb2 == 2 else 0
                        gT = n0 + n1
                        msg = msg_pool.tile([128, gT * 256], f8e3, tag="msg")
                        msg3 = msg[:].rearrange("p (c e) -> p c e", e=256)
                        off2 = plan.offB2[b0]
                        tbl2 = ag_out[2].rearrange("(r h) e -> r (h e)", h=4)
                        nc.gpsimd.dma_gather(
                            msg3[:, :, :], tbl2,
                            idxU_t[:, off2 * 8:(off2 + gT) * 8],
                            num_idxs=gT * 128, num_idxs_reg=gT * 128,
                            elem_size=256, single_packet=False,
                            queue_num=g % 4)
                        sdt = f8e3
                        doff = off2
                        runs = {b0: (0, n0)}
                        if nb2 == 2:
                            runs[b0 + 1] = (n0, n1)

                    S = s_pool.tile([128, gT * 128], sdt, tag="S")
                    S3 = S[:].rearrange("p (c d) -> p c d", d=128)
                    dlt = dstloc_t if l < 2 else dstloc2_t
                    dl3 = (dlt[:, doff:doff + gT]
                           .rearrange("p (c o) -> p c o", o=1)
                           .to_broadcast([128, gT, 128]))
                    io3 = (iota[:].rearrange("p (o d) -> p o d", o=1)
                           .to_broadcast([128, gT, 128]))
                    nc.vector.tensor_tensor(
                        out=S3, in0=dl3, in1=io3, op=_mb.AluOpType.is_equal)
                    return msg, msg3, S, runs

                def phase3_combine(b, grp):
                    msg, msg3, S, runs = grp
                    if l < 2:
                        bruns = runs[b]
                        nT = sum(n for _, n in bruns)
                    else:
                        c0, nT = runs[b]
                        clsb = plan.cls2[b]

                    if nT:
                        pma = pma_pool.tile([128, dout], f32, tag="pma")
                        done = 0
                        if l < 2:
                            # fp8 DoubleRow over chunk pairs within each run
                            for k0, n in bruns:
                                npair = n // 2
                                for j in range(npair):
                                    k = k0 + 2 * j
                                    nc.tensor.matmul(
                                        pma[:],
                                        lhsT=S[:, k * 128:(k + 2) * 128]
                                        .rearrange("p (two d) -> p two d", two=2),
                                        rhs=msg[:, k * dout:(k + 2) * dout]
                                        .rearrange("p (two e) -> p two e", two=2),
                                        start=(done == 0 and j == 0),
                                        stop=(done + 2 * (j + 1) == nT and n % 2 == 0),
                                        perf_mode=_mb.MatmulPerfMode.DoubleRow)
                                if n % 2:
                                    k = k0 + n - 1
                                    nc.tensor.matmul(
                                        pma[:], lhsT=S[:, k * 128:(k + 1) * 128],
                                        rhs=msg[:, k * dout:(k + 1) * dout],
                                        start=(done == 0 and n == 1),
                                        stop=(done + n == nT))
                                done += n
                        else:
                            for j in range(nT):
                                k = c0 + j
                                c4 = clsb[j]
                                nc.tensor.matmul(
                                    pma[:], lhsT=S[:, k * 128:(k + 1) * 128],
                                    rhs=msg3[:, k, 64 * c4:64 * c4 + 64],
                                    start=(j == 0), stop=(j == nT - 1))

                        aggS = a_pool.tile([128, dout], f16, tag="aggS")
                        dsc = deginv_t if l < 2 else deginv8_t
                        nc.scalar.activation(
                            aggS[:], pma[:],
                            _mb.ActivationFunctionType.Copy,
                            scale=dsc[:, b:b + 1])

                    # combine: pm = deginv*agg + root_b
                    pm = pm_pool.tile([128, dout], f32, tag="pm")
                    if nT:
                        nc.tensor.matmul(pm[:], lhsT=ident[:],
                                         rhs=aggS[:], start=True, stop=False)
                    nc.tensor.matmul(pm[:], lhsT=ident[:],
                                     rhs=rootS[:, b * dout:(b + 1) * dout],
                                     start=(nT == 0), stop=True)
                    return pm

                def phase3_tail(b, pm):
                    bs = slice(b * 128, (b + 1) * 128)
                    rows = min(128, nsh - b * 128)
                    if l < 2:
                        hsb = h_pool.tile([128, dout], f16, tag="h")
                        nc.scalar.activation(
                            hsb[:], pm[:],
                            _mb.ActivationFunctionType.Relu)
                        if "notr" not in dbg:
                            for q in range(kc):
                                ptr = tr_pool.tile([128, 128], f16, tag="tr")
                                nc.tensor.transpose(ptr[:], hsb[:, q * 128:(q + 1) * 128],
                                                    ident[:])
                                nc.scalar.copy(hT3[:, q, bs], ptr[:])
                        # fused next-layer phase 1 on the freshly written block
                        phase1_block(l + 1, b)
                    else:
                        osb = o_pool.tile([128, dout], f32, tag="o")
                        nc.scalar.copy(osb[:], pm[:])
                        nc.sync.dma_start(out_d[b * 128:b * 128 + rows, :],
                                          osb[:rows, :])

                prev = None
                prev_b = None
                for g in range((nblk + 1) // 2):
                    grp = gather_group(g)
                    for b in range(2 * g, min(2 * g + 2, nblk)):
                        cur = phase3_combine(b, grp)
                        if prev is not None:
                            phase3_tail(prev_b, prev)
                        prev, prev_b = cur, b
                phase3_tail(prev_b, prev)

    nc.compile()
    return nc


# ---------------------------------------------------------------------------
# Entry point
# ---------------------------------------------------------------------------

LAST_RESULTS = None
_CACHE = {}


def _run(x, edge_index, weights, n_nodes, n_cores, d_in, d_hid, d_out,
         low_lim, trace=False):
    global LAST_RESULTS
    from concourse.bass_utils import run_bass_kernel_spmd

    plan, per_core = preprocess(x, edge_index, n_nodes, n_cores, d_in, low_lim)
    fp = (n_nodes, d_in, d_hid, d_out, tuple(plan.nL), tuple(plan.nH))
    if fp not in _CACHE:
        _CACHE[fp] = build_program(plan, d_in, d_hid, d_out)
    nc = _CACHE[fp]

    const = {}
    for l, (Wl, Wr, b) in enumerate(weights):
        const[f"wl{l}"] = np.asarray(Wl, np.float32).astype(np.float16)
        const[f"wr{l}"] = np.asarray(Wr, np.float32).astype(np.float16)
        const[f"b{l}"] = np.asarray(b, np.float32).astype(np.float16)[None, :]

    in_maps = []
    for c in range(n_cores):
        m = dict(const)
        pc = per_core[c]
        m["xT"] = pc["xT"]
        m["idxL"] = pc["idxL"] if plan.CL else np.zeros((128, 1), np.int16)
        m["idxH"] = pc["idxH"] if plan.CH else np.zeros((128, 1), np.int16)
        m["dstloc"] = pc["dstloc"]
        m["deginv"] = pc["deginv"]
        m["idx2"] = pc["idx2"] if plan.TOT2 else np.zeros((128, 1), np.int16)
        m["dstloc2"] = pc["dstloc2"]
        m["deginv8"] = pc["deginv8"]
        in_maps.append(m)

    res = run_bass_kernel_spmd(nc, in_maps, core_ids=list(range(n_cores)),
                               trace=trace)
    LAST_RESULTS = res
    out = np.concatenate([res.results[c]["out"] for c in range(n_cores)], axis=0)
    return out.astype(np.float32)


def kernel(x, edge_index, relations=None, Wl0=None, Wr0=None, b0=None,
           Wl1=None, Wr1=None, b1=None, Wl2=None, Wr2=None, b2=None,
           **kw):
    x = np.asarray(x, np.float32)
    edge_index = np.asarray(edge_index)
    weights = [(Wl0, Wr0, b0), (Wl1, Wr1, b1), (Wl2, Wr2, b2)]
    import os
    trace = bool(int(os.environ.get("KERNEL_TRACE", "0")))
    return _run(x, edge_index, weights, N_NODES, N_CORES, D_IN, D_HID, D_OUT,
                LOW_LIM_FULL, trace=trace)



# revision 43
# speedup vs baseline: 1.0018x; 1.0018x over previous
"""3-layer GraphSAGE (ClusterGCN-style) on 8 Trainium2 NeuronCores.

Strategy (graph/data parallel, transform-first):
  - Nodes sharded by contiguous range across 8 cores (6250 each).
  - Per layer l: t = h @ Wl computed on own shard (fused into the previous
    layer's per-block epilogue) -> AllGather t -> per 128-dst block:
    dma_gather the incoming edges' t[src] rows -> segment-sum via one-hot
    matmul on the tensor engine -> combine with the root path h @ Wr + b
    (precomputed into SBUF during the AllGather window, when every engine
    would otherwise idle) -> relu -> PE transposes back to feature-major,
    in place in the single h table.
  - Layers 0/1 tables are fp8e4 (halves the AllGather bytes and the gather
    traffic; 512B rows keep full DMA rate). Aggregation uses pure-0/1 fp8
    one-hot S with MatmulPerfMode.DoubleRow over chunk pairs (256-deep
    contraction at 0.5 cycles/row); deg_inv is applied exactly via a
    per-partition fp32 scale on the PSUM->SBUF copy, and the root is added
    with an identity matmul.
  - Layer 2 (64-wide) packs 4 nodes per 256B fp8e3 table row (values
    pre-scaled by 8 to stay in e3m4's normal range), satisfying
    dma_gather's 256B row-alignment with a 4x smaller table; chunk class
    src%4 selects the 64-col slice of each gathered row.
  - Edges are dst-sorted on host into 128-edge chunks per (block, class)
    with static chunk counts (max over cores -> one SPMD program); gathers
    and the S build are batched over pairs of blocks and spread across 4
    SWDGE queues; the per-block epilogue is software-pipelined one block
    behind the aggregation.
  - fp32 PSUM accumulation everywhere; fp32 output.
"""

import math
import numpy as np

N_NODES = 50000
N_EDGES = 800000
D_IN = 512
D_HID = 512
D_OUT = 64
N_CORES = 8
LOW_LIM_FULL = 32768


# ---------------------------------------------------------------------------
# Host preprocessing
# ---------------------------------------------------------------------------

class Plan:
    pass


def _wrap_idx(v):
    """Pack an index vector (len multiple of 16) into the [16, m/16]
    pattern dma_gather expects, replicated to 128 partitions."""
    a = np.asarray(v, np.int16).reshape(-1, 16).T  # [16, m/16]
    return np.tile(a, (8, 1))  # [128, m/16]


def preprocess(x, edge_index, n_nodes, n_cores, d_in, low_lim):
    """Returns (plan, per_core_inputs_list)."""
    src = np.asarray(edge_index[0], np.int64)
    dst = np.asarray(edge_index[1], np.int64)
    nsh = n_nodes // n_cores
    nblk = math.ceil(nsh / 128)
    nfree = nblk * 128
    kc = d_in // 128

    deg = np.bincount(dst, minlength=n_nodes).astype(np.float32)
    deginv = (1.0 / np.maximum(deg, 1.0)).astype(np.float32)

    core = dst // nsh
    bid = core * nblk + (dst - core * nsh) // 128
    cls = (src >= low_lim).astype(np.int64)
    order = np.lexsort((dst, cls, bid))
    src_s = src[order]
    dst_s = dst[order]
    bid_s = bid[order]
    cls_s = cls[order]

    key = bid_s * 2 + cls_s
    ngrp = n_cores * nblk * 2
    starts = np.searchsorted(key, np.arange(ngrp + 1), side="left")

    # static chunk counts per (block, class): max over cores
    cnt = (starts[1:] - starts[:-1]).reshape(n_cores, nblk, 2)
    nchunk = -(-cnt // 128)  # ceil
    nL = nchunk[:, :, 0].max(axis=0)  # [nblk]
    nH = nchunk[:, :, 1].max(axis=0)
    CL = int(nL.sum())
    CH = int(nH.sum())
    TOTC = CL + CH
    offL = np.concatenate([[0], np.cumsum(nL)[:-1]]).astype(np.int64)
    offH = np.concatenate([[0], np.cumsum(nH)[:-1]]).astype(np.int64)
    offT = np.concatenate([[0], np.cumsum(nL + nH)[:-1]]).astype(np.int64)

    plan = Plan()
    plan.nsh, plan.nblk, plan.nfree, plan.kc = nsh, nblk, nfree, kc
    plan.nL, plan.nH = nL.tolist(), nH.tolist()
    plan.CL, plan.CH, plan.TOTC = CL, CH, TOTC
    plan.offL, plan.offH, plan.offT = offL.tolist(), offH.tolist(), offT.tolist()
    plan.low_lim = low_lim
    plan.n_cores = n_cores
    plan.n_nodes = n_nodes

    # ---- layer-2 packed-table structure: 4 nodes per 256B fp8 row, so the
    # 64-wide final layer satisfies dma_gather's 256B alignment with a table
    # 4x smaller. Chunk class = src%4 selects the 64-col slice of the row.
    core_src = src // nsh
    ls = src - core_src * nsh
    row4 = core_src * (nfree // 4) + ls // 4  # row in the [8*nfree/4, 256] view
    cls4 = ls % 4
    order2 = np.lexsort((dst, cls4, bid))
    row_s2 = row4[order2]
    dst_s2 = dst[order2]
    cls_s2 = cls4[order2]
    bid_s2 = bid[order2]
    key2 = bid_s2 * 4 + cls_s2
    starts2 = np.searchsorted(key2, np.arange(n_cores * nblk * 4 + 1), "left")
    cnt2 = (starts2[1:] - starts2[:-1]).reshape(n_cores, nblk, 4)
    n4 = (-(-cnt2 // 128)).max(axis=0)  # [nblk, 4]
    TOT2 = int(n4.sum())
    offB2 = np.concatenate([[0], np.cumsum(n4.sum(axis=1))[:-1]]).astype(np.int64)
    plan.n4 = n4.tolist()
    plan.TOT2 = TOT2
    plan.offB2 = offB2.tolist()
    plan.cls2 = [sum(([c] * int(n4[b, c]) for c in range(4)), [])
                 for b in range(nblk)]

    x = np.asarray(x, np.float32)
    per_core = []
    for c in range(n_cores):
        # layer-2 packed-table chunk inputs
        idx2 = np.zeros((TOT2 * 128,), np.int64)
        dloc2 = np.full((TOT2 * 128,), -1.0, np.float32)
        for b in range(nblk):
            pos = offB2[b]
            for cl in range(4):
                g = (c * nblk + b) * 4 + cl
                s0, s1 = starts2[g], starts2[g + 1]
                n_e = s1 - s0
                nX = n4[b][cl]
                if nX:
                    seg = idx2[pos * 128:(pos + nX) * 128]
                    seg[:n_e] = row_s2[s0:s1]
                    dseg = dloc2[pos * 128:(pos + nX) * 128]
                    dseg[:n_e] = (dst_s2[s0:s1] - c * nsh - b * 128).astype(
                        np.float32)
                    pos += nX
        wi_2 = _wrap_idx(idx2) if TOT2 else np.zeros((128, 0), np.int16)
        dl2 = dloc2.reshape(TOT2, 128).T.astype(np.float16)  # [128, TOT2]

        idxL = np.zeros((CL * 128,), np.int64)
        idxH = np.zeros((CH * 128,), np.int64)
        dloc = np.full((TOTC * 128,), -1.0, np.float32)
        for b in range(nblk):
            for t, (nX, offX, idxX, base) in enumerate(
                ((nL[b], offL[b], idxL, 0), (nH[b], offH[b], idxH, low_lim))
            ):
                g = (c * nblk + b) * 2 + t
                s0, s1 = starts[g], starts[g + 1]
                n_e = s1 - s0
                if nX == 0:
                    continue
                seg = idxX[offX * 128:(offX + nX) * 128]
                seg[:n_e] = src_s[s0:s1] - base
                # dloc columns in gather-group order (pairs of blocks):
                # [L(b0) L(b1) H(b0) H(b1)]
                b0 = b - (b % 2)
                b1 = b0 + 1
                nLb1 = nL[b1] if b1 < nblk else 0
                if t == 0:
                    dof = (offT[b0] + (0 if b == b0 else nL[b0])) * 128
                else:
                    dof = (offT[b0] + nL[b0] + nLb1
                           + (0 if b == b0 else nH[b0])) * 128
                dseg = dloc[dof:dof + nX * 128]
                dseg[:n_e] = (dst_s[s0:s1] - c * nsh - b * 128).astype(np.float32)
        wi_L = _wrap_idx(idxL) if CL else np.zeros((128, 0), np.int16)
        wi_H = _wrap_idx(idxH) if CH else np.zeros((128, 0), np.int16)
        # dloc layout must match gather output: edge i -> partition i%128,
        # chunk i//128; dstloc[p, chunk] = dloc of that edge
        dl = dloc.reshape(TOTC, 128).T.astype(np.float16)  # [128, TOTC]

        # per-partition deginv columns: dg_b[p, b] = deginv of node b*128+p
        dg = np.zeros((nfree,), np.float32)
        dg[:nsh] = deginv[c * nsh:(c + 1) * nsh]
        dg_b = np.ascontiguousarray(
            dg.reshape(nblk, 128).T.astype(np.float32))  # [128, nblk]

        xT = np.zeros((kc, 128, nfree), np.float16)
        xs = x[c * nsh:(c + 1) * nsh]  # [nsh, d_in]
        xT[:, :, :nsh] = xs.T.reshape(kc, 128, nsh)

        per_core.append({
            "xT": xT,
            "idxL": np.ascontiguousarray(wi_L),
            "idxH": np.ascontiguousarray(wi_H),
            "dstloc": np.ascontiguousarray(dl),
            "deginv": dg_b,
            "idx2": np.ascontiguousarray(wi_2),
            "dstloc2": np.ascontiguousarray(dl2),
            "deginv8": dg_b / 8.0,
        })
    return plan, per_core


# ---------------------------------------------------------------------------
# Device program
# ---------------------------------------------------------------------------

def build_program(plan, d_in, d_hid, d_out):
    import os
    dbg = set(os.environ.get("KDBG", "").split(",")) - {""}
    import concourse.bacc as bacc
    import concourse.tile as tile
    from concourse import bass, mybir
    from concourse.masks import make_identity

    f16 = mybir.dt.float16
    f32 = mybir.dt.float32
    f8 = mybir.dt.float8e4
    f8e3 = mybir.dt.float8e3
    i16 = mybir.dt.int16

    nsh, nblk, nfree = plan.nsh, plan.nblk, plan.nfree
    kcs = [d_in // 128, d_hid // 128, d_hid // 128]
    douts = [d_hid, d_hid, d_out]
    n_cores = plan.n_cores
    n_nodes = plan.n_nodes
    CL, CH, TOTC = plan.CL, plan.CH, plan.TOTC
    low_lim = plan.low_lim
    kc0 = kcs[0]

    nc = bacc.Bacc(
        "TRN2",
        target_bir_lowering=False,
        debug=False,
        num_devices=n_cores,
        num_swdge_queues=4,
    )

    xT_d = nc.dram_tensor("xT", [kc0, 128, nfree], f16, kind="ExternalInput").ap()
    w_d = {}
    for l in range(3):
        kd = [d_in, d_hid, d_hid][l]
        w_d[(l, "l")] = nc.dram_tensor(f"wl{l}", [kd, douts[l]], f16,
                                       kind="ExternalInput").ap()
        w_d[(l, "r")] = nc.dram_tensor(f"wr{l}", [kd, douts[l]], f16,
                                       kind="ExternalInput").ap()
    b_d = [nc.dram_tensor(f"b{l}", [1, douts[l]], f16, kind="ExternalInput").ap()
           for l in range(3)]
    idxL_d = nc.dram_tensor("idxL", [128, max(CL * 8, 1)], i16,
                            kind="ExternalInput").ap()
    idxH_d = nc.dram_tensor("idxH", [128, max(CH * 8, 1)], i16,
                            kind="ExternalInput").ap()
    dstloc_d = nc.dram_tensor("dstloc", [128, TOTC], f16, kind="ExternalInput").ap()
    deginv_d = nc.dram_tensor("deginv", [128, nblk], f32, kind="ExternalInput").ap()
    TOT2 = plan.TOT2
    idx2_d = nc.dram_tensor("idx2", [128, max(TOT2 * 8, 1)], i16,
                            kind="ExternalInput").ap()
    dstloc2_d = nc.dram_tensor("dstloc2", [128, TOT2], f16,
                               kind="ExternalInput").ap()
    deginv8_d = nc.dram_tensor("deginv8", [128, nblk], f32,
                               kind="ExternalInput").ap()
    out_d = nc.dram_tensor("out", [nsh, d_out], f32, kind="ExternalOutput").ap()

    with tile.TileContext(nc) as tc:
        # --- DRAM bounce buffers for the AllGathers
        ag_in, ag_out = [], []
        for l in range(3):
            if l < 2:
                ai = nc.dram_tensor(f"agi{l}", [nsh, douts[l]], f8,
                                    kind="Internal").ap()
                ao = nc.dram_tensor(f"ago{l}", [n_nodes, douts[l]], f8,
                                    kind="Internal", addr_space="Shared").ap()
            else:
                # packed: 4 nodes per 256B row (viewed [nfree,64] for writes,
                # [n_cores*nfree//4, 256] for gathers); values pre-scaled by 8
                # so fp8e3 stays in its normal range
                ai = nc.dram_tensor(f"agi{l}", [nfree, douts[l]], f8e3,
                                    kind="Internal").ap()
                ao = nc.dram_tensor(f"ago{l}", [n_cores * nfree, douts[l]],
                                    f8e3, kind="Internal",
                                    addr_space="Shared").ap()
            ag_in.append(ai)
            ag_out.append(ao)

        import contextlib
        with contextlib.ExitStack() as ctx:
            nb = 1 if "serial" in dbg else 3
            cpool = ctx.enter_context(tc.tile_pool(name="const", bufs=1))
            pt_pool = ctx.enter_context(
                tc.tile_pool(name="pt", bufs=2, space="PSUM"))
            pm_pool = ctx.enter_context(
                tc.tile_pool(name="pm", bufs=2, space="PSUM"))
            pma_pool = ctx.enter_context(
                tc.tile_pool(name="pma", bufs=2, space="PSUM"))
            tr_pool = ctx.enter_context(
                tc.tile_pool(name="tr", bufs=2, space="PSUM"))
            tsb_pool = ctx.enter_context(tc.tile_pool(name="tsb", bufs=nb))
            msg_pool = ctx.enter_context(tc.tile_pool(name="msg", bufs=2))
            s_pool = ctx.enter_context(tc.tile_pool(name="spool", bufs=3))
            a_pool = ctx.enter_context(tc.tile_pool(name="apool", bufs=3))
            h_pool = ctx.enter_context(tc.tile_pool(name="hpool", bufs=3))
            o_pool = ctx.enter_context(tc.tile_pool(name="opool", bufs=3))

            # --- constants
            # single in-place h table: layer l+1's rows overwrite layer l's
            # block by block after the root path has consumed them
            hT = cpool.tile([128, kc0 * nfree], f16, name="hT")
            hT3 = hT[:].rearrange("p (q n) -> p q n", n=nfree)
            xT3 = xT_d.rearrange("q p n -> p q n")
            step = (nblk // 4) * 128
            for c0 in range(0, nfree, step):
                c1 = min(c0 + step, nfree)
                nc.sync.dma_start(hT3[:, :, c0:c1], xT3[:, :, c0:c1])
            # root-path results (h @ Wr + b), staged in SBUF so the PE can
            # fill the AllGather window with this work
            rootS = cpool.tile([128, nblk * max(douts)], f16, name="rootS")

            ident = cpool.tile([128, 128], f16, name="ident")
            make_identity(nc, ident[:])
            iota = cpool.tile([128, 128], f16, name="iota")
            nc.gpsimd.iota(iota[:], pattern=[[1, 128]], base=0,
                           channel_multiplier=0,
                           allow_small_or_imprecise_dtypes=True)
            ones = cpool.tile([1, 128], f16, name="ones")
            nc.vector.memset(ones[:], 1.0)

            wt = {}
            for l in range(3):
                kd = kcs[l]
                for s in ("l", "r"):
                    t = cpool.tile([128, kd * douts[l]], f16, name=f"w{s}{l}")
                    nc.sync.dma_start(
                        t[:].rearrange("p (q d) -> p q d", d=douts[l]),
                        w_d[(l, s)].rearrange("(q p) d -> p q d", p=128))
                    wt[(l, s)] = t
            bt = []
            for l in range(3):
                t = cpool.tile([1, douts[l]], f16, name=f"bt{l}")
                nc.sync.dma_start(t[:], b_d[l][:, :])
                bt.append(t)

            IDXW = max(CL * 8 + CH * 8, TOT2 * 8, 1)
            idxU_t = cpool.tile([128, IDXW], i16, name="idxUt")
            nc.sync.dma_start(idxU_t[:, :CL * 8], idxL_d[:, :])
            nc.sync.dma_start(idxU_t[:, CL * 8:CL * 8 + CH * 8], idxH_d[:, :])
            dstloc_t = cpool.tile([128, TOTC], f16, name="dstloct")
            nc.sync.dma_start(dstloc_t[:], dstloc_d[:, :])
            deginv_t = cpool.tile([128, nblk], f32, name="deginvt")
            nc.sync.dma_start(deginv_t[:], deginv_d[:, :])
            dstloc2_t = cpool.tile([128, TOT2], f16, name="dstloc2t")
            nc.sync.dma_start(dstloc2_t[:], dstloc2_d[:, :])
            deginv8_t = cpool.tile([128, nblk], f32, name="deginv8t")
            nc.sync.dma_start(deginv8_t[:], deginv8_d[:, :])

            rg = [list(range(n_cores))]
            from concourse import mybir as _mb

            def phase1_block(l, b):
                """t = h @ Wl for one block -> ag_in[l] (feeds the AllGather)."""
                dout = douts[l]
                kc = kcs[l]
                tdt = f8 if l < 2 else f8e3
                bs = slice(b * 128, (b + 1) * 128)
                rows = min(128, nsh - b * 128)
                pt = pt_pool.tile([128, dout], f32, tag="pt")
                for q in range(kc):
                    nc.tensor.matmul(
                        pt[:], lhsT=hT3[:, q, bs],
                        rhs=wt[(l, "l")][:, q * dout:(q + 1) * dout],
                        start=(q == 0), stop=(q == kc - 1))
                tsb = tsb_pool.tile([128, dout], tdt, tag="tsb")
                if l < 2:
                    nc.scalar.copy(tsb[:], pt[:])
                    nc.sync.dma_start(ag_in[l][b * 128:b * 128 + rows, :],
                                      tsb[:rows, :])
                else:
                    nc.scalar.activation(tsb[:], pt[:],
                                         _mb.ActivationFunctionType.Copy,
                                         scale=8.0)
                    nc.sync.dma_start(ag_in[l][b * 128:(b + 1) * 128, :],
                                      tsb[:, :])

            # ---- layer 0 phase 1 (from the x table)
            for b in range(nblk):
                phase1_block(0, b)

            for l in range(3):
                dout = douts[l]
                kc = kcs[l]
                tdt = f8 if l < 2 else f32

                # ---- AllGather t
                if "nocoll" not in dbg:
                    nc.gpsimd.collective_compute(
                        "AllGather", _mb.AluOpType.bypass, replica_groups=rg,
                        ins=[ag_in[l].opt()], outs=[ag_out[l].opt()])

                if l == 2:
                    # swap in the layer-2 gather indices (layers 0/1 are done
                    # with the union tile by now)
                    nc.sync.dma_start(idxU_t[:, :TOT2 * 8], idx2_d[:, :])

                # ---- root path (h @ Wr + b) -> rootS; no dependency on the
                # AllGather, so the PE runs this inside the collective window
                for b in range(nblk):
                    bs = slice(b * 128, (b + 1) * 128)
                    pmr = pt_pool.tile([128, dout], f32, tag="pt")
                    for q in range(kc):
                        nc.tensor.matmul(
                            pmr[:], lhsT=hT3[:, q, bs],
                            rhs=wt[(l, "r")][:, q * dout:(q + 1) * dout],
                            start=(q == 0), stop=False)
                    nc.tensor.matmul(pmr[:], lhsT=ones[:1, :], rhs=bt[l][:1, :],
                                     start=False, stop=True)
                    nc.scalar.copy(rootS[:, b * dout:(b + 1) * dout], pmr[:])

                # ---- phase 3, software-pipelined by one block:
                # head(b) = gathers/S/agg/combine, tail(b) = relu/transpose/
                # fused next phase1 (or output store). tail(b-1) is emitted
                # after head(b) so no engine queue waits on the relu round
                # trip between PE and Act.
                def gather_group(g):
                    """One msg tile + batched gathers + one S build covering
                    the pair of blocks (2g, 2g+1)."""
                    b0 = 2 * g
                    nb2 = 2 if b0 + 1 < nblk else 1
                    if l < 2:
                        nL0 = plan.nL[b0]
                        nH0 = plan.nH[b0]
                        nL1 = plan.nL[b0 + 1] if nb2 == 2 else 0
                        nH1 = plan.nH[b0 + 1] if nb2 == 2 else 0
                        nLg = nL0 + nL1
                        nHg = nH0 + nH1
                        gT = nLg + nHg
                        msg = msg_pool.tile([128, gT * dout], tdt, tag="msg")
                        msg3 = msg[:].rearrange("p (c e) -> p c e", e=dout)
                        if nLg:
                            nc.gpsimd.dma_gather(
                                msg3[:, :nLg, :], ag_out[l][:, :],
                                idxU_t[:, plan.offL[b0] * 8:(plan.offL[b0] + nLg) * 8],
                                num_idxs=nLg * 128, num_idxs_reg=nLg * 128,
                                elem_size=dout, single_packet=False,
                                queue_num=(2 * g) % 4)
                        if nHg:
                            nc.gpsimd.dma_gather(
                                msg3[:, nLg:, :], ag_out[l][low_lim:, :],
                                idxU_t[:, (CL + plan.offH[b0]) * 8:(CL + plan.offH[b0] + nHg) * 8],
                                num_idxs=nHg * 128, num_idxs_reg=nHg * 128,
                                elem_size=dout, single_packet=False,
                                queue_num=(2 * g + 1) % 4)
                        sdt = f8
                        doff = plan.offT[b0]
                        # per-block chunk runs in the group msg layout
                        runs = {b0: [(0, nL0), (nLg, nH0)]}
                        if nb2 == 2:
                            runs[b0 + 1] = [(nL0, nL1), (nLg + nH0, nH1)]
                    else:
                        n0 = sum(plan.n4[b0])
                        n1 = sum(plan.n4[b0 + 1]) if nb2 == 2 else 0
                        gT = n0 + n1
                        msg = msg_pool.tile([128, gT * 256], f8e3, tag="msg")
                        msg3 = msg[:].rearrange("p (c e) -> p c e", e=256)
                        off2 = plan.offB2[b0]
                        tbl2 = ag_out[2].rearrange("(r h) e -> r (h e)", h=4)
                        nc.gpsimd.dma_gather(
                            msg3[:, :, :], tbl2,
                            idxU_t[:, off2 * 8:(off2 + gT) * 8],
                            num_idxs=gT * 128, num_idxs_reg=gT * 128,
                            elem_size=256, single_packet=False,
                            queue_num=g % 4)
                        sdt = f8e3
                        doff = off2
                        runs = {b0: (0, n0)}
                        if nb2 == 2:
                            runs[b0 + 1] = (n0, n1)

                    S = s_pool.tile([128, gT * 128], sdt, tag="S")
                    S3 = S[:].rearrange("p (c d) -> p c d", d=128)
                    dlt = dstloc_t if l < 2 else dstloc2_t
                    dl3 = (dlt[:, doff:doff + gT]
                           .rearrange("p (c o) -> p c o", o=1)
                           .to_broadcast([128, gT, 128]))
                    io3 = (iota[:].rearrange("p (o d) -> p o d", o=1)
                           .to_broadcast([128, gT, 128]))
                    nc.vector.tensor_tensor(
                        out=S3, in0=dl3, in1=io3, op=_mb.AluOpType.is_equal)
                    return msg, msg3, S, runs

                def phase3_combine(b, grp):
                    msg, msg3, S, runs = grp
                    if l < 2:
                        bruns = runs[b]
                        nT = sum(n for _, n in bruns)
                    else:
                        c0, nT = runs[b]
                        clsb = plan.cls2[b]

                    if nT:
                        pma = pma_pool.tile([128, dout], f32, tag="pma")
                        done = 0
                        if l < 2:
                            # fp8 DoubleRow over chunk pairs within each run
                            for k0, n in bruns:
                                npair = n // 2
                                for j in range(npair):
                                    k = k0 + 2 * j
                                    nc.tensor.matmul(
                                        pma[:],
                                        lhsT=S[:, k * 128:(k + 2) * 128]
                                        .rearrange("p (two d) -> p two d", two=2),
                                        rhs=msg[:, k * dout:(k + 2) * dout]
                                        .rearrange("p (two e) -> p two e", two=2),
                                        start=(done == 0 and j == 0),
                                        stop=(done + 2 * (j + 1) == nT and n % 2 == 0),
                                        perf_mode=_mb.MatmulPerfMode.DoubleRow)
                                if n % 2:
                                    k = k0 + n - 1
                                    nc.tensor.matmul(
                                        pma[:], lhsT=S[:, k * 128:(k + 1) * 128],
                                        rhs=msg[:, k * dout:(k + 1) * dout],
                                        start=(done == 0 and n == 1),
                                        stop=(done + n == nT))
                                done += n
                        else:
                            for j in range(nT):
                                k = c0 + j
                                c4 = clsb[j]
                                nc.tensor.matmul(
                                    pma[:], lhsT=S[:, k * 128:(k + 1) * 128],
                                    rhs=msg3[:, k, 64 * c4:64 * c4 + 64],
                                    start=(j == 0), stop=(j == nT - 1))

                        aggS = a_pool.tile([128, dout], f16, tag="aggS")
                        dsc = deginv_t if l < 2 else deginv8_t
                        nc.scalar.activation(
                            aggS[:], pma[:],
                            _mb.ActivationFunctionType.Copy,
                            scale=dsc[:, b:b + 1])

                    # combine: pm = deginv*agg + root_b
                    pm = pm_pool.tile([128, dout], f32, tag="pm")
                    if nT:
                        nc.tensor.matmul(pm[:], lhsT=ident[:],
                                         rhs=aggS[:], start=True, stop=False)
                    nc.tensor.matmul(pm[:], lhsT=ident[:],
                                     rhs=rootS[:, b * dout:(b + 1) * dout],
                                     start=(nT == 0), stop=True)
                    return pm

                def phase3_tail(b, pm):
                    bs = slice(b * 128, (b + 1) * 128)
                    rows = min(128, nsh - b * 128)
                    if l < 2:
                        hsb = h_pool.tile([128, dout], f16, tag="h")
                        nc.scalar.activation(
                            hsb[:], pm[:],
                            _mb.ActivationFunctionType.Relu)
                        if "notr" not in dbg:
                            for q in range(kc):
                                ptr = tr_pool.tile([128, 128], f16, tag="tr")
                                nc.tensor.transpose(ptr[:], hsb[:, q * 128:(q + 1) * 128],
                                                    ident[:])
                                nc.scalar.copy(hT3[:, q, bs], ptr[:])
                        # fused next-layer phase 1 on the freshly written block
                        phase1_block(l + 1, b)
                    else:
                        osb = o_pool.tile([128, dout], f32, tag="o")
                        nc.scalar.copy(osb[:], pm[:])
                        nc.sync.dma_start(out_d[b * 128:b * 128 + rows, :],
                                          osb[:rows, :])

                prev = None
                prev_b = None
                for g in range((nblk + 1) // 2):
                    grp = gather_group(g)
                    for b in range(2 * g, min(2 * g + 2, nblk)):
                        cur = phase3_combine(b, grp)
                        if prev is not None:
                            phase3_tail(prev_b, prev)
                        prev, prev_b = cur, b
                phase3_tail(prev_b, prev)

    nc.compile()
    return nc


# ---------------------------------------------------------------------------
# Entry point
# ---------------------------------------------------------------------------

LAST_RESULTS = None
_CACHE = {}


def _run(x, edge_index, weights, n_nodes, n_cores, d_in, d_hid, d_out,
         low_lim, trace=False):
    global LAST_RESULTS
    from concourse.bass_utils import run_bass_kernel_spmd

    plan, per_core = preprocess(x, edge_index, n_nodes, n_cores, d_in, low_lim)
    fp = (n_nodes, d_in, d_hid, d_out, tuple(plan.nL), tuple(plan.nH))
    if fp not in _CACHE:
        _CACHE[fp] = build_program(plan, d_in, d_hid, d_out)
    nc = _CACHE[fp]

    const = {}
    for l, (Wl, Wr, b) in enumerate(weights):
        const[f"wl{l}"] = np.asarray(Wl, np.float32).astype(np.float16)
        const[f"wr{l}"] = np.asarray(Wr, np.float32).astype(np.float16)
        const[f"b{l}"] = np.asarray(b, np.float32).astype(np.float16)[None, :]

    in_maps = []
    for c in range(n_cores):
        m = dict(const)
        pc = per_core[c]
        m["xT"] = pc["xT"]
        m["idxL"] = pc["idxL"] if plan.CL else np.zeros((128, 1), np.int16)
        m["idxH"] = pc["idxH"] if plan.CH else np.zeros((128, 1), np.int16)
        m["dstloc"] = pc["dstloc"]
        m["deginv"] = pc["deginv"]
        m["idx2"] = pc["idx2"] if plan.TOT2 else np.zeros((128, 1), np.int16)
        m["dstloc2"] = pc["dstloc2"]
        m["deginv8"] = pc["deginv8"]
        in_maps.append(m)

    res = run_bass_kernel_spmd(nc, in_maps, core_ids=list(range(n_cores)),
                               trace=trace)
    LAST_RESULTS = res
    out = np.concatenate([res.results[c]["out"] for c in range(n_cores)], axis=0)
    return out.astype(np.float32)


def kernel(x, edge_index, relations=None, Wl0=None, Wr0=None, b0=None,
           Wl1=None, Wr1=None, b1=None, Wl2=None, Wr2=None, b2=None,
           **kw):
    x = np.asarray(x, np.float32)
    edge_index = np.asarray(edge_index)
    weights = [(Wl0, Wr0, b0), (Wl1, Wr1, b1), (Wl2, Wr2, b2)]
    import os
    trace = bool(int(os.environ.get("KERNEL_TRACE", "0")))
    return _run(x, edge_index, weights, N_NODES, N_CORES, D_IN, D_HID, D_OUT,
                LOW_LIM_FULL, trace=trace)

